# revision 13
# baseline (speedup 1.0000x reference)
"""Multi-head attention (B=2, S=2048, D=1024, H=16) on 8 TRN2 NeuronCores.

Sharding: core c handles batch c//4 and heads 4*(c%4) .. 4*(c%4)+4
(tensor-parallel over heads x data-parallel over batch).

Per-core pipeline (all matmuls bf16 with fp32 PSUM accumulation):
  1. qT/kT = W @ X^T   [d=256 on partitions, s=2048 free]  (transposed proj)
     v     = X @ W^T   [s on partitions, d free] + ones column (for softmax den)
  2. Per head pair (row-packed on the PE array, contraction d=64):
     S^T[k,q] = kT.T @ qT   -> PSUM, exp(S/8) on ScalarE -> p^T bf16 in SBUF
  3. out^T[d,q] = sum_k [v|1].T @ p^T  (M=65: row 64 = softmax denominator)
     scale by reciprocal denominator (gpsimd partition-broadcast + DVE)
  4. partial[s, 1024] = out^T.T @ WoT  (only this core's head block of W_o)
Host: full output[b] = sum of the 4 partials for batch b + b_o.
"""
import numpy as np
import ml_dtypes

import concourse.bass as bass
import concourse.mybir as mybir
from concourse.tile import TileContext as _TileContext
from concourse.vector_clock import ScopedClock
from concourse.bass_utils import run_bass_kernel_spmd


TileContext = _TileContext
_ = ScopedClock  # (kept import for potential debugging)


def split_multi_waits(nc):
    """This container's walrus codegen allows only one sync-wait command per
    instruction ("Too many sync wait commands" in setupSyncWait). Tile
    sometimes attaches several semaphore waits to one instruction; hoist the
    extras onto dedicated EventSemaphore instructions inserted immediately
    before, on the same engine (sequencers execute in order, so semantics
    are identical)."""
    n = [0]
    for f in nc.m.functions:
        for blk in f.blocks:
            new_insts = []
            changed = False
            for inst in blk.instructions:
                si = inst.sync_info
                if si is not None and len(si.on_wait) > 1:
                    waits = list(si.on_wait)
                    for w in waits[:-1]:
                        n[0] += 1
                        ev = mybir.InstEventSemaphore(
                            name=f"WSPLIT-{n[0]}",
                            ins=[], outs=[],
                        )
                        ev.engine = inst.engine
                        ev.sync_info = mybir.SyncInfo(on_wait=[w], on_update=[])
                        new_insts.append(ev)
                        nc.register_instruction(ev, overwrite=True)
                    inst.sync_info = mybir.SyncInfo(
                        on_wait=[waits[-1]], on_update=list(si.on_update)
                    )
                    changed = True
                new_insts.append(inst)
            if changed:
                blk.instructions = new_insts
    return n[0]

BF16 = mybir.dt.bfloat16
F32 = mybir.dt.float32
F32R = mybir.dt.float32r

B, S, D = 2, 2048, 1024
H, DK = 16, 64
HPC = 4              # heads per core
DC = HPC * DK        # 256 d-model dims per core
N_CORES = 8
P = 128              # partitions
SB = S // P          # 16 s-blocks
FC = D // P          # 8 feature chunks
QSUP = 512           # q tile width for attention
NQ = S // QSUP       # 2 q supertiles
QC = QSUP // 512     # 512-wide matmul chunks per q supertile


def build_bass():
    nc = bass.Bass()
    # DRAM inputs (per-core shard, prepared on host)
    xtq = nc.dram_tensor("xtq", [D, S], BF16, kind="ExternalInput")
    xtk = nc.dram_tensor("xtk", [D, S], BF16, kind="ExternalInput")
    xtv = nc.dram_tensor("xtv", [D, S], BF16, kind="ExternalInput")
    wqt = nc.dram_tensor("wqt", [D, DC], BF16, kind="ExternalInput")
    wkt = nc.dram_tensor("wkt", [D, DC], BF16, kind="ExternalInput")
    wvt = nc.dram_tensor("wvt", [D, DC], BF16, kind="ExternalInput")
    wot = nc.dram_tensor("wot", [DC, D], BF16, kind="ExternalInput")
    bq = nc.dram_tensor("bq", [DC, 1], F32, kind="ExternalInput")
    bk = nc.dram_tensor("bk", [DC, 1], F32, kind="ExternalInput")
    bvr = nc.dram_tensor("bvr", [1, DC], F32, kind="ExternalInput")
    outp = nc.dram_tensor("outp", [S, D], F32, kind="ExternalOutput")

    with TileContext(nc) as tc:
        consts = tc.alloc_tile_pool(name="consts", bufs=1)
        qkv = tc.alloc_tile_pool(name="qkv", bufs=1)
        psp = tc.alloc_tile_pool(name="psp", bufs=1, space="PSUM")

        # ---- load constants ----
        wq_sb = consts.tile([P, FC, DC], BF16, tag="wq")
        wk_sb = consts.tile([P, FC, DC], BF16, tag="wk")
        wv_sb = consts.tile([P, FC, DC], BF16, tag="wv")
        nc.sync.dma_start(wq_sb[:], wqt.rearrange("(c p) d -> p c d", p=P))
        nc.sync.dma_start(wk_sb[:], wkt.rearrange("(c p) d -> p c d", p=P))
        nc.sync.dma_start(wv_sb[:], wvt.rearrange("(c p) d -> p c d", p=P))
        wo_sb = consts.tile([P, DC // P, D], BF16, tag="wo")
        nc.sync.dma_start(wo_sb[:], wot.rearrange("(c p) n -> p c n", p=P))
        bq_sb = consts.tile([P, DC // P, 1], F32, tag="bq")
        bk_sb = consts.tile([P, DC // P, 1], F32, tag="bk")
        nc.sync.dma_start(bq_sb[:], bq.rearrange("(c p) o -> p c o", p=P))
        nc.sync.dma_start(bk_sb[:], bk.rearrange("(c p) o -> p c o", p=P))
        bv_row = consts.tile([1, DC], F32, tag="bvrow")
        nc.sync.dma_start(bv_row[:], bvr[:])
        ones_f32 = consts.tile([1, P], F32, tag="ones_f32")
        nc.vector.memset(ones_f32[:], 1.0)
        ones_row = consts.tile([1, P], F32R, tag="ones_row")
        nc.vector.tensor_copy(ones_row[:], ones_f32[:])
        # replicate bv across partitions: ones[1,128].T @ bv_row[1,256] (K=1 matmul)
        bv_r = consts.tile([1, DC], F32R, tag="bv_r")
        nc.vector.tensor_copy(bv_r[:], bv_row[:])
        bv_rep = consts.tile([P, DC], F32, tag="bvrep")
        bv_ps = psp.tile([P, DC], F32, tag="bv_ps")
        nc.tensor.matmul(bv_ps[:], ones_row[:], bv_r[:], start=True, stop=True)
        nc.vector.tensor_copy(bv_rep[:], bv_ps[:])

        # ---- projections ----
        # qT/kT: [dc, s] with d on partitions (2 chunks), via lhsT=W^T, rhs=X^T
        qt_sb = qkv.tile([P, DC // P, S], BF16, tag="qt")
        kt_sb = qkv.tile([P, DC // P, S], BF16, tag="kt")
        # v natural [s, d] + ones col per head: [P, sblk, head, 65]
        v_sb = qkv.tile([P, SB, HPC, DK + 1], BF16, tag="v")
        nc.vector.memset(v_sb[:, :, :, DK:], 1.0)

        with tc.tile_pool(name="xt", bufs=1) as xt_pool:
            xq_sb = xt_pool.tile([P, FC, S], BF16, tag="xq")
            xk_sb = xt_pool.tile([P, FC, S], BF16, tag="xk")
            xv_sb = xt_pool.tile([P, FC, S], BF16, tag="xv")
            nc.sync.dma_start(xq_sb[:], xtq.rearrange("(c p) s -> p c s", p=P))
            nc.sync.dma_start(xk_sb[:], xtk.rearrange("(c p) s -> p c s", p=P))
            nc.sync.dma_start(xv_sb[:], xtv.rearrange("(c p) s -> p c s", p=P))

            # qT / kT projections
            for name, x_sb, w_sb, b_sb, t_sb in (
                ("q", xq_sb, wq_sb, bq_sb, qt_sb),
                ("k", xk_sb, wk_sb, bk_sb, kt_sb),
            ):
                for dc in range(DC // P):
                    for ss in range(S // 512):
                        ps = psp.tile([P, 512], F32, tag="proj_ps")
                        for fc in range(FC):
                            nc.tensor.matmul(
                                ps[:],
                                w_sb[:, fc, dc * P:(dc + 1) * P],
                                x_sb[:, fc, ss * 512:(ss + 1) * 512],
                                start=(fc == 0), stop=(fc == FC - 1),
                            )
                        nc.vector.tensor_scalar(
                            t_sb[:, dc, ss * 512:(ss + 1) * 512],
                            ps[:], b_sb[:, dc, :], None, mybir.AluOpType.add,
                        )
            # v projection (natural layout)
            for sb_i in range(SB):
                ps = psp.tile([P, DC], F32, tag="vproj_ps")
                for fc in range(FC):
                    nc.tensor.matmul(
                        ps[:],
                        x_sb_v_slice := xv_sb[:, fc, sb_i * P:(sb_i + 1) * P],
                        wv_sb[:, fc, :],
                        start=(fc == 0), stop=(fc == FC - 1),
                    )
                nc.vector.tensor_tensor(
                    v_sb[:, sb_i, :, :DK],
                    ps[:].rearrange("p (h d) -> p h d", h=HPC),
                    bv_rep[:].rearrange("p (h d) -> p h d", h=HPC),
                    mybir.AluOpType.add,
                )

        psp.release()

        # ---- attention ----
        # p^T tile: [k=128, head-in-pair 2, kblk 16, q QSUP] bf16
        pt_pool = tc.alloc_tile_pool(name="pt", bufs=2)
        dyn = tc.alloc_tile_pool(name="dyn", bufs=2)
        att_ps = tc.alloc_tile_pool(name="att_ps", bufs=1, space="PSUM")
        sc_pool = tc.alloc_tile_pool(name="sc_pool", bufs=2, space="PSUM")
        outt_sb = qkv.tile([P, 2, S], BF16, tag="outt")  # [d-in-pair, pair, q]

        for pair in range(2):            # heads (2*pair, 2*pair+1), d-chunk=pair
            for qs in range(NQ):
                q0 = qs * QSUP
                pt = pt_pool.tile([P, 2, SB, QSUP], BF16, tag="pt")
                av_ps = [
                    att_ps.tile([DK + 1, QSUP], F32, tag=f"av{hh}", name=f"av{hh}")
                    for hh in range(2)
                ]
                for kb in range(SB):
                    sc_ps = sc_pool.tile([P, 2, QSUP], F32, tag="sc")
                    for hh in range(2):          # head-in-pair -> partition 64*hh
                        hp = hh * DK
                        for qc in range(QC):
                            nc.tensor.matmul(
                                sc_ps[:, hh, qc * 512:(qc + 1) * 512],
                                kt_sb[hp:hp + DK, pair, kb * P:(kb + 1) * P],
                                qt_sb[hp:hp + DK, pair,
                                      q0 + qc * 512:q0 + (qc + 1) * 512],
                                start=True, stop=True,
                            )
                    # exp(score/8) -> p^T (both heads in one ACT op)
                    nc.scalar.activation(
                        pt[:, :, kb, :], sc_ps[:],
                        mybir.ActivationFunctionType.Exp,
                        bias=0.0, scale=0.125,
                    )
                    # A·V accumulation ([v|1]: row 64 = denominator)
                    for hh in range(2):
                        h = 2 * pair + hh
                        for qc in range(QC):
                            nc.tensor.matmul(
                                av_ps[hh][:, qc * 512:(qc + 1) * 512],
                                v_sb[:, kb, h, :],
                                pt[:, hh, kb, qc * 512:(qc + 1) * 512],
                                start=(kb == 0), stop=(kb == SB - 1),
                            )
                # normalize: out^T = av[0:64] * (1/av[64]);
                # the reciprocal row is replicated across partitions with a
                # K=1 ones matmul (f32r runs at full PE rate for N>=256)
                rec = dyn.tile([1, 2, QSUP], F32, tag="rec")
                rec_r = dyn.tile([1, 2, QSUP], F32R, tag="rec_r")
                rec_ps = att_ps.tile([P, 2, QSUP], F32, tag="rec_ps")
                for hh in range(2):
                    nc.vector.reciprocal(rec[:, hh, :], av_ps[hh][DK:, :])
                    nc.vector.tensor_copy(rec_r[:, hh, :], rec[:, hh, :])
                    nc.tensor.matmul(
                        rec_ps[:, hh, :], ones_row[:], rec_r[:, hh, :],
                        start=True, stop=True,
                    )
                rec_rep = dyn.tile([P, 2, QSUP], F32, tag="rec_rep")
                nc.vector.tensor_copy(rec_rep[:], rec_ps[:])
                for hh in range(2):
                    nc.vector.tensor_tensor(
                        outt_sb[hh * DK:(hh + 1) * DK, pair, q0:q0 + QSUP],
                        av_ps[hh][:DK, :],
                        rec_rep[hh * DK:(hh + 1) * DK, hh, :],
                        mybir.AluOpType.mult,
                    )

        # ---- output projection: partial[s, 1024] ----
        sc_pool.release()
        att_ps.release()
        opr_ps = tc.alloc_tile_pool(name="opr_ps", bufs=2, space="PSUM")
        for sb_i in range(SB):
            ps = opr_ps.tile([P, D], F32, tag="oproj_ps")
            for pair in range(2):
                for ncnk in range(D // 512):
                    nc.tensor.matmul(
                        ps[:, ncnk * 512:(ncnk + 1) * 512],
                        outt_sb[:, pair, sb_i * P:(sb_i + 1) * P],
                        wo_sb[:, pair, ncnk * 512:(ncnk + 1) * 512],
                        start=(pair == 0), stop=(pair == 1),
                    )
            o_sb = dyn.tile([P, D], F32, tag="o_out")
            nc.vector.tensor_copy(o_sb[:], ps[:])
            nc.sync.dma_start(outp[sb_i * P:(sb_i + 1) * P, :], o_sb[:])

        for pool in (opr_ps, dyn, pt_pool, qkv, consts):
            pool.release()  # LIFO within each memory space

    n_split = split_multi_waits(nc)
    return nc


_NC_CACHE = None


def kernel(Q, K, V, W_q, b_q, W_k, b_k, W_v, b_v, W_o, b_o):
    global _NC_CACHE
    bf = ml_dtypes.bfloat16
    Q, K, V = np.asarray(Q), np.asarray(K), np.asarray(V)

    # host-side shard prep
    xt = {}   # per batch: transposed bf16 inputs
    for b in range(B):
        xt[b] = (
            np.ascontiguousarray(Q[b].T).astype(bf),
            np.ascontiguousarray(K[b].T).astype(bf),
            np.ascontiguousarray(V[b].T).astype(bf),
        )
    in_maps = []
    for c in range(N_CORES):
        b = c // 4
        g = c % 4
        sl = slice(g * DC, (g + 1) * DC)
        in_maps.append({
            "xtq": xt[b][0], "xtk": xt[b][1], "xtv": xt[b][2],
            "wqt": np.ascontiguousarray(np.asarray(W_q)[sl, :].T).astype(bf),
            "wkt": np.ascontiguousarray(np.asarray(W_k)[sl, :].T).astype(bf),
            "wvt": np.ascontiguousarray(np.asarray(W_v)[sl, :].T).astype(bf),
            "wot": np.ascontiguousarray(np.asarray(W_o)[:, sl].T).astype(bf),
            "bq": np.asarray(b_q)[sl].reshape(DC, 1).astype(np.float32),
            "bk": np.asarray(b_k)[sl].reshape(DC, 1).astype(np.float32),
            "bvr": np.asarray(b_v)[sl].reshape(1, DC).astype(np.float32),
        })

    if _NC_CACHE is None:
        _NC_CACHE = build_bass()
    res = run_bass_kernel_spmd(_NC_CACHE, in_maps, core_ids=list(range(N_CORES)))

    out = np.zeros((B, S, D), np.float32)
    for c in range(N_CORES):
        out[c // 4] += res.results[c]["outp"]
    out += np.asarray(b_o).astype(np.float32)
    return out


# revision 20
# speedup vs baseline: 1.9266x; 1.9266x over previous
"""Multi-head attention (B=2, S=2048, D=1024, H=16) on 8 TRN2 NeuronCores.

Sharding: core c handles batch c//4 and heads 4*(c%4) .. 4*(c%4)+4
(tensor-parallel over heads x data-parallel over batch).

Per-core pipeline (all matmuls bf16 with fp32 PSUM accumulation):
  1. qT/kT = W @ X^T   [d=256 on partitions, s=2048 free]  (transposed proj)
     v     = X @ W^T   [s on partitions, d free] + ones column (for softmax den)
  2. Per head pair (row-packed on the PE array, contraction d=64):
     S^T[k,q] = kT.T @ qT   -> PSUM, exp(S/8) on ScalarE -> p^T bf16 in SBUF
  3. out^T[d,q] = sum_k [v|1].T @ p^T  (M=65: row 64 = softmax denominator)
     scale by reciprocal denominator (gpsimd partition-broadcast + DVE)
  4. partial[s, 1024] = out^T.T @ WoT  (only this core's head block of W_o)
Host: full output[b] = sum of the 4 partials for batch b + b_o.
"""
import numpy as np
import ml_dtypes

import concourse.bass as bass
import concourse.mybir as mybir
from concourse.tile import TileContext as _TileContext
from concourse.vector_clock import ScopedClock
from concourse.bass_utils import run_bass_kernel_spmd


TileContext = _TileContext
_ = ScopedClock  # (kept import for potential debugging)


def split_multi_waits(nc):
    """This container's walrus codegen allows only one sync-wait command per
    instruction ("Too many sync wait commands" in setupSyncWait). Tile
    sometimes attaches several semaphore waits to one instruction; hoist the
    extras onto dedicated EventSemaphore instructions inserted immediately
    before, on the same engine (sequencers execute in order, so semantics
    are identical)."""
    n = [0]
    for f in nc.m.functions:
        for blk in f.blocks:
            new_insts = []
            changed = False
            for inst in blk.instructions:
                si = inst.sync_info
                if si is not None and len(si.on_wait) > 1:
                    waits = list(si.on_wait)
                    for w in waits[:-1]:
                        n[0] += 1
                        ev = mybir.InstEventSemaphore(
                            name=f"WSPLIT-{n[0]}",
                            ins=[], outs=[],
                        )
                        ev.engine = inst.engine
                        ev.sync_info = mybir.SyncInfo(on_wait=[w], on_update=[])
                        new_insts.append(ev)
                        nc.register_instruction(ev, overwrite=True)
                    inst.sync_info = mybir.SyncInfo(
                        on_wait=[waits[-1]], on_update=list(si.on_update)
                    )
                    changed = True
                new_insts.append(inst)
            if changed:
                blk.instructions = new_insts
    return n[0]

BF16 = mybir.dt.bfloat16
F32 = mybir.dt.float32
F32R = mybir.dt.float32r

B, S, D = 2, 2048, 1024
H, DK = 16, 64
HPC = 4              # heads per core
DC = HPC * DK        # 256 d-model dims per core
N_CORES = 8
P = 128              # partitions
SB = S // P          # 16 s-blocks
FC = D // P          # 8 feature chunks
QSUP = 512           # q tile width for attention
NQ = S // QSUP       # 2 q supertiles
QC = QSUP // 512     # 512-wide matmul chunks per q supertile


def build_bass():
    nc = bass.Bass()
    # DRAM inputs (per-core shard, prepared on host)
    xtq = nc.dram_tensor("xtq", [D, S], BF16, kind="ExternalInput")
    xtk = nc.dram_tensor("xtk", [D, S], BF16, kind="ExternalInput")
    xtv = nc.dram_tensor("xtv", [D, S], BF16, kind="ExternalInput")
    wqt = nc.dram_tensor("wqt", [D, DC], BF16, kind="ExternalInput")
    wkt = nc.dram_tensor("wkt", [D, DC], BF16, kind="ExternalInput")
    wvt = nc.dram_tensor("wvt", [D, DC], BF16, kind="ExternalInput")
    wot = nc.dram_tensor("wot", [DC, D], BF16, kind="ExternalInput")
    bq = nc.dram_tensor("bq", [DC, 1], F32, kind="ExternalInput")
    bk = nc.dram_tensor("bk", [DC, 1], F32, kind="ExternalInput")
    bvr = nc.dram_tensor("bvr", [1, DC], F32, kind="ExternalInput")
    outp = nc.dram_tensor("outp", [S, D], F32, kind="ExternalOutput")

    with TileContext(nc) as tc:
        # SBUF pool stack (LIFO releases): consts, qkv, ptpool | xt_v, xt_kq
        # PSUM pool stack: sc_pool, att_ps | psp, then opr_ps
        consts = tc.alloc_tile_pool(name="consts", bufs=1)
        qkv = tc.alloc_tile_pool(name="qkv", bufs=1)
        ptpool = tc.alloc_tile_pool(name="ptpool", bufs=2)
        dyn = tc.alloc_tile_pool(name="dyn", bufs=1)
        opool = tc.alloc_tile_pool(name="opool", bufs=2)
        sc_pool = tc.alloc_tile_pool(name="sc_pool", bufs=2, space="PSUM")
        att_ps = tc.alloc_tile_pool(name="att_ps", bufs=1, space="PSUM")
        psp = tc.alloc_tile_pool(name="psp", bufs=1, space="PSUM")

        # ---- constants (k path first: it gates the critical path) ----
        wk_sb = consts.tile([P, FC, DC], BF16, tag="wk")
        wq_sb = consts.tile([P, FC, DC], BF16, tag="wq")
        bq_sb = consts.tile([P, DC // P, 1], F32, tag="bq")
        bk_sb = consts.tile([P, DC // P, 1], F32, tag="bk")
        wo_sb = consts.tile([P, DC // P, D], BF16, tag="wo")
        nc.sync.dma_start(wk_sb[:], wkt.rearrange("(c p) d -> p c d", p=P))
        nc.sync.dma_start(bk_sb[:], bk.rearrange("(c p) o -> p c o", p=P))
        ones_f32 = consts.tile([1, P], F32, tag="ones_f32")
        nc.vector.memset(ones_f32[:], 1.0)
        ones_row = consts.tile([1, P], F32R, tag="ones_row")
        nc.vector.tensor_copy(ones_row[:], ones_f32[:])

        # ---- persistent activations ----
        qt_sb = qkv.tile([P, DC // P, S], BF16, tag="qt")
        kt_sb = qkv.tile([P, DC // P, S], BF16, tag="kt")
        # v natural [s, d] + ones col per head: [P, sblk, head, 65]
        v_sb = qkv.tile([P, SB, HPC, DK + 1], BF16, tag="v")
        nc.vector.memset(v_sb[:, :, :, DK:], 1.0)
        outt_sb = qkv.tile([P, 2, S], BF16, tag="outt")  # [d-in-pair, pair, q]

        def scores_exp(pair, qs, pt):
            """S^T matmuls (row-packed head pair) + exp into pt."""
            q0 = qs * QSUP
            for kb in range(SB):
                sc_ps = sc_pool.tile([P, 2, QSUP], F32, tag="sc", name="sc")
                for hh in range(2):          # head-in-pair -> partitions 64*hh
                    hp = hh * DK
                    for qc in range(QC):
                        nc.tensor.matmul(
                            sc_ps[:, hh, qc * 512:(qc + 1) * 512],
                            kt_sb[hp:hp + DK, pair, kb * P:(kb + 1) * P],
                            qt_sb[hp:hp + DK, pair,
                                  q0 + qc * 512:q0 + (qc + 1) * 512],
                            start=True, stop=True,
                        )
                nc.scalar.activation(
                    pt[:, :, kb, :], sc_ps[:],
                    mybir.ActivationFunctionType.Exp,
                    bias=0.0, scale=0.125,
                )

        def av_normalize(pair, qs, pt):
            """A·V ([v|1]: row 64 = denominator), then scale by 1/den."""
            q0 = qs * QSUP
            av_ps = [
                att_ps.tile([DK + 1, QSUP], F32, tag=f"av{hh}", name=f"av{hh}")
                for hh in range(2)
            ]
            for kb in range(SB):
                for hh in range(2):
                    h = 2 * pair + hh
                    for qc in range(QC):
                        nc.tensor.matmul(
                            av_ps[hh][:, qc * 512:(qc + 1) * 512],
                            v_sb[:, kb, h, :],
                            pt[:, hh, kb, qc * 512:(qc + 1) * 512],
                            start=(kb == 0), stop=(kb == SB - 1),
                        )
            # reciprocal denominator, replicated across partitions via a
            # K=1 ones matmul (f32r runs at full PE rate for N>=256)
            rec = dyn.tile([1, 2, QSUP], F32, tag="rec", name="rec")
            rec_r = dyn.tile([1, 2, QSUP], F32R, tag="rec_r", name="rec_r")
            rec_ps = sc_pool.tile([P, 2, QSUP], F32, tag="sc", name="rec_ps")
            for hh in range(2):
                nc.vector.reciprocal(rec[:, hh, :], av_ps[hh][DK:, :])
                nc.vector.tensor_copy(rec_r[:, hh, :], rec[:, hh, :])
                nc.tensor.matmul(
                    rec_ps[:, hh, :], ones_row[:], rec_r[:, hh, :],
                    start=True, stop=True,
                )
            rec_rep = dyn.tile([P, 2, QSUP], F32, tag="rec_rep", name="rec_rep")
            nc.vector.tensor_copy(rec_rep[:], rec_ps[:])
            for hh in range(2):
                nc.vector.tensor_tensor(
                    outt_sb[hh * DK:(hh + 1) * DK, pair, q0:q0 + QSUP],
                    av_ps[hh][:DK, :],
                    rec_rep[hh * DK:(hh + 1) * DK, hh, :],
                    mybir.AluOpType.mult,
                )

        def o_proj(qs):
            """Output projection + store for the s-range covered by qs."""
            for sb_i in range(qs * QSUP // P, (qs + 1) * QSUP // P):
                for ncnk in range(D // 512):
                    ps = opr_ps.tile([P, 512], F32, tag="oproj_ps", name="ps")
                    for pair in range(2):
                        nc.tensor.matmul(
                            ps[:],
                            outt_sb[:, pair, sb_i * P:(sb_i + 1) * P],
                            wo_sb[:, pair, ncnk * 512:(ncnk + 1) * 512],
                            start=(pair == 0), stop=(pair == 1),
                        )
                    o_sb = opool.tile([P, 512], F32, tag="o_out", name="o_sb")
                    nc.vector.tensor_copy(o_sb[:], ps[:])
                    nc.sync.dma_start(
                        outp[sb_i * P:(sb_i + 1) * P,
                             ncnk * 512:(ncnk + 1) * 512],
                        o_sb[:],
                    )

        def qk_proj(x_sb, w_sb, b_sb, t_sb, dc):
            for ss in range(S // 512):
                ps = psp.tile([P, 512], F32, tag="proj_ps", name="ps")
                for fc in range(FC):
                    nc.tensor.matmul(
                        ps[:],
                        w_sb[:, fc, dc * P:(dc + 1) * P],
                        x_sb[:, fc, ss * 512:(ss + 1) * 512],
                        start=(fc == 0), stop=(fc == FC - 1),
                    )
                nc.vector.tensor_scalar(
                    t_sb[:, dc, ss * 512:(ss + 1) * 512],
                    ps[:], b_sb[:, dc, :], None, mybir.AluOpType.add,
                )

        # ---- phase A: load X_k/X_q (chunked), project k/q, and emit the
        # first two attention blocks (pair 0 only needs d-chunk 0) so
        # ScalarE has ~37us of exp work covering the remaining projections
        with tc.tile_pool(name="xt_kq", bufs=1) as xt_kq:
            xk_sb = xt_kq.tile([P, FC, S], BF16, tag="xk")
            xq_sb = xt_kq.tile([P, FC, S], BF16, tag="xq")
            srck = xtk.rearrange("(c p) s -> p c s", p=P)
            for fc in range(FC):
                nc.sync.dma_start(xk_sb[:, fc, :], srck[:, fc, :])
            nc.sync.dma_start(wq_sb[:], wqt.rearrange("(c p) d -> p c d", p=P))
            nc.sync.dma_start(bq_sb[:], bq.rearrange("(c p) o -> p c o", p=P))
            srcq = xtq.rearrange("(c p) s -> p c s", p=P)
            for fc in range(FC):
                nc.sync.dma_start(xq_sb[:, fc, :], srcq[:, fc, :])
            nc.sync.dma_start(wo_sb[:], wot.rearrange("(c p) n -> p c n", p=P))

            qk_proj(xk_sb, wk_sb, bk_sb, kt_sb, 0)
            qk_proj(xq_sb, wq_sb, bq_sb, qt_sb, 0)
            pt0 = ptpool.tile([P, 2, SB, QSUP], BF16, tag="pt", name="pt0")
            scores_exp(0, 0, pt0)
            pt1 = ptpool.tile([P, 2, SB, QSUP], BF16, tag="pt", name="pt1")
            scores_exp(0, 1, pt1)
            qk_proj(xk_sb, wk_sb, bk_sb, kt_sb, 1)
            qk_proj(xq_sb, wq_sb, bq_sb, qt_sb, 1)

        # ---- phase B: load X_v, project v (+bias via K=1 broadcast) ----
        with tc.tile_pool(name="xt_v", bufs=1) as xt_v:
            wv_sb = xt_v.tile([P, FC, DC], BF16, tag="wv")
            nc.sync.dma_start(wv_sb[:], wvt.rearrange("(c p) d -> p c d", p=P))
            bv_row = xt_v.tile([1, DC], F32, tag="bvrow")
            nc.sync.dma_start(bv_row[:], bvr[:])
            bv_r = xt_v.tile([1, DC], F32R, tag="bv_r")
            nc.vector.tensor_copy(bv_r[:], bv_row[:])
            bv_rep = xt_v.tile([P, DC], F32, tag="bvrep")
            bv_ps = psp.tile([P, DC], F32, tag="vproj_ps")
            nc.tensor.matmul(bv_ps[:], ones_row[:], bv_r[:], start=True, stop=True)
            nc.vector.tensor_copy(bv_rep[:], bv_ps[:])

            xv_sb = xt_v.tile([P, FC, S], BF16, tag="xv")
            srcv = xtv.rearrange("(c p) s -> p c s", p=P)
            for fc in range(FC):
                nc.sync.dma_start(xv_sb[:, fc, :], srcv[:, fc, :])
            for sb_i in range(SB):
                ps = psp.tile([P, DC], F32, tag="vproj_ps", name="ps")
                for fc in range(FC):
                    nc.tensor.matmul(
                        ps[:],
                        xv_sb[:, fc, sb_i * P:(sb_i + 1) * P],
                        wv_sb[:, fc, :],
                        start=(fc == 0), stop=(fc == FC - 1),
                    )
                nc.vector.tensor_tensor(
                    v_sb[:, sb_i, :, :DK],
                    ps[:].rearrange("p (h d) -> p h d", h=HPC),
                    bv_rep[:].rearrange("p (h d) -> p h d", h=HPC),
                    mybir.AluOpType.add,
                )
            # drain pair (0, qs=0) inside this phase so the pt slot frees
            # before the main loop's first scores block needs it
            av_normalize(0, 0, pt0)

        psp.release()
        opr_ps = tc.alloc_tile_pool(name="opr_ps", bufs=2, space="PSUM")

        # ---- main pipeline: blocks (0,0) and (0,1) already emitted ----
        # iteration order: pair-0 first two (only need d-chunk 0), then
        # interleaved so each qs's O-projection fires as soon as both pairs
        # finish. pending holds blocks whose A·V tail is not yet emitted.
        rest = [(0, 1), (1, 1), (2, 0), (2, 1), (3, 0), (3, 1)]
        pending = [(1, 0, pt1)]
        done_pairs = {(0, 0): True}  # A·V emitted at the end of phase B
        for qs, pair in rest:
            pt = ptpool.tile([P, 2, SB, QSUP], BF16, tag="pt", name="pt")
            scores_exp(pair, qs, pt)
            pqs, ppair, ppt = pending.pop(0)
            av_normalize(ppair, pqs, ppt)
            done_pairs[(pqs, ppair)] = True
            if (pqs, 0) in done_pairs and (pqs, 1) in done_pairs:
                o_proj(pqs)
            pending.append((qs, pair, pt))
        for pqs, ppair, ppt in pending:
            av_normalize(ppair, pqs, ppt)
            done_pairs[(pqs, ppair)] = True
            if (pqs, 0) in done_pairs and (pqs, 1) in done_pairs:
                o_proj(pqs)
        for pool in (opr_ps, att_ps, sc_pool, opool, dyn, ptpool, qkv, consts):
            pool.release()  # LIFO within each memory space

    n_split = split_multi_waits(nc)
    return nc


_NC_CACHE = None


def kernel(Q, K, V, W_q, b_q, W_k, b_k, W_v, b_v, W_o, b_o):
    global _NC_CACHE
    bf = ml_dtypes.bfloat16
    Q, K, V = np.asarray(Q), np.asarray(K), np.asarray(V)

    # host-side shard prep
    xt = {}   # per batch: transposed bf16 inputs
    for b in range(B):
        xt[b] = (
            np.ascontiguousarray(Q[b].T).astype(bf),
            np.ascontiguousarray(K[b].T).astype(bf),
            np.ascontiguousarray(V[b].T).astype(bf),
        )
    in_maps = []
    for c in range(N_CORES):
        b = c // 4
        g = c % 4
        sl = slice(g * DC, (g + 1) * DC)
        in_maps.append({
            "xtq": xt[b][0], "xtk": xt[b][1], "xtv": xt[b][2],
            "wqt": np.ascontiguousarray(np.asarray(W_q)[sl, :].T).astype(bf),
            "wkt": np.ascontiguousarray(np.asarray(W_k)[sl, :].T).astype(bf),
            "wvt": np.ascontiguousarray(np.asarray(W_v)[sl, :].T).astype(bf),
            "wot": np.ascontiguousarray(np.asarray(W_o)[:, sl].T).astype(bf),
            "bq": np.asarray(b_q)[sl].reshape(DC, 1).astype(np.float32),
            "bk": np.asarray(b_k)[sl].reshape(DC, 1).astype(np.float32),
            "bvr": np.asarray(b_v)[sl].reshape(1, DC).astype(np.float32),
        })

    if _NC_CACHE is None:
        _NC_CACHE = build_bass()
    res = run_bass_kernel_spmd(_NC_CACHE, in_maps, core_ids=list(range(N_CORES)))

    out = np.zeros((B, S, D), np.float32)
    for c in range(N_CORES):
        out[c // 4] += res.results[c]["outp"]
    out += np.asarray(b_o).astype(np.float32)
    return out


# revision 21
# speedup vs baseline: 332.8795x; 172.7786x over previous
"""Multi-head attention (B=2, S=2048, D=1024, H=16) on 8 TRN2 NeuronCores.

Sharding: core c handles batch c//4 and heads 4*(c%4) .. 4*(c%4)+4
(tensor-parallel over heads x data-parallel over batch).

Per-core pipeline (all matmuls bf16 with fp32 PSUM accumulation):
  1. qT/kT = W @ X^T   [d=256 on partitions, s=2048 free]  (transposed proj)
     v     = X @ W^T   [s on partitions, d free] + ones column (for softmax den)
  2. Per head pair (row-packed on the PE array, contraction d=64):
     S^T[k,q] = kT.T @ qT   -> PSUM, exp(S/8) on ScalarE -> p^T bf16 in SBUF
  3. out^T[d,q] = sum_k [v|1].T @ p^T  (M=65: row 64 = softmax denominator)
     scale by reciprocal denominator (gpsimd partition-broadcast + DVE)
  4. partial[s, 1024] = out^T.T @ WoT  (only this core's head block of W_o)
Host: full output[b] = sum of the 4 partials for batch b + b_o.
"""
import numpy as np
import ml_dtypes

import concourse.bass as bass
import concourse.mybir as mybir
from concourse.tile import TileContext as _TileContext
from concourse.vector_clock import ScopedClock
from concourse.bass_utils import run_bass_kernel_spmd


TileContext = _TileContext
_ = ScopedClock  # (kept import for potential debugging)


def split_multi_waits(nc):
    """This container's walrus codegen allows only one sync-wait command per
    instruction ("Too many sync wait commands" in setupSyncWait). Tile
    sometimes attaches several semaphore waits to one instruction; hoist the
    extras onto dedicated EventSemaphore instructions inserted immediately
    before, on the same engine (sequencers execute in order, so semantics
    are identical)."""
    n = [0]
    for f in nc.m.functions:
        for blk in f.blocks:
            new_insts = []
            changed = False
            for inst in blk.instructions:
                si = inst.sync_info
                if si is not None and len(si.on_wait) > 1:
                    waits = list(si.on_wait)
                    for w in waits[:-1]:
                        n[0] += 1
                        ev = mybir.InstEventSemaphore(
                            name=f"WSPLIT-{n[0]}",
                            ins=[], outs=[],
                        )
                        ev.engine = inst.engine
                        ev.sync_info = mybir.SyncInfo(on_wait=[w], on_update=[])
                        new_insts.append(ev)
                        nc.register_instruction(ev, overwrite=True)
                    inst.sync_info = mybir.SyncInfo(
                        on_wait=[waits[-1]], on_update=list(si.on_update)
                    )
                    changed = True
                new_insts.append(inst)
            if changed:
                blk.instructions = new_insts
    return n[0]

BF16 = mybir.dt.bfloat16
F32 = mybir.dt.float32
F32R = mybir.dt.float32r

B, S, D = 2, 2048, 1024
H, DK = 16, 64
HPC = 4              # heads per core
DC = HPC * DK        # 256 d-model dims per core
N_CORES = 8
P = 128              # partitions
SB = S // P          # 16 s-blocks
FC = D // P          # 8 feature chunks
QSUP = 512           # q tile width for attention
NQ = S // QSUP       # 2 q supertiles
QC = QSUP // 512     # 512-wide matmul chunks per q supertile


def build_bass():
    nc = bass.Bass()
    # DRAM inputs (per-core shard, prepared on host)
    xtq = nc.dram_tensor("xtq", [D, S], BF16, kind="ExternalInput")
    xtk = nc.dram_tensor("xtk", [D, S], BF16, kind="ExternalInput")
    xtv = nc.dram_tensor("xtv", [D, S], BF16, kind="ExternalInput")
    wqt = nc.dram_tensor("wqt", [D, DC], BF16, kind="ExternalInput")
    wkt = nc.dram_tensor("wkt", [D, DC], BF16, kind="ExternalInput")
    wvt = nc.dram_tensor("wvt", [D, DC], BF16, kind="ExternalInput")
    wot = nc.dram_tensor("wot", [DC, D], BF16, kind="ExternalInput")
    bq = nc.dram_tensor("bq", [DC, 1], F32, kind="ExternalInput")
    bk = nc.dram_tensor("bk", [DC, 1], F32, kind="ExternalInput")
    bvr = nc.dram_tensor("bvr", [1, DC], F32, kind="ExternalInput")
    outp = nc.dram_tensor("outp", [S, D], F32, kind="ExternalOutput")

    with TileContext(nc) as tc:
        # SBUF pool stack (LIFO releases): consts, qkv, ptpool | xt_v, xt_kq
        # PSUM pool stack: sc_pool, att_ps | psp, then opr_ps
        consts = tc.alloc_tile_pool(name="consts", bufs=1)
        qkv = tc.alloc_tile_pool(name="qkv", bufs=1)
        ptpool = tc.alloc_tile_pool(name="ptpool", bufs=2)
        dyn = tc.alloc_tile_pool(name="dyn", bufs=1)
        opool = tc.alloc_tile_pool(name="opool", bufs=2)
        sc_pool = tc.alloc_tile_pool(name="sc_pool", bufs=2, space="PSUM")
        att_ps = tc.alloc_tile_pool(name="att_ps", bufs=1, space="PSUM")
        psp = tc.alloc_tile_pool(name="psp", bufs=1, space="PSUM")

        # ---- constants (k path first: it gates the critical path) ----
        wk_sb = consts.tile([P, FC, DC], BF16, tag="wk")
        wq_sb = consts.tile([P, FC, DC], BF16, tag="wq")
        bq_sb = consts.tile([P, DC // P, 1], F32, tag="bq")
        bk_sb = consts.tile([P, DC // P, 1], F32, tag="bk")
        wo_sb = consts.tile([P, DC // P, D], BF16, tag="wo")
        nc.sync.dma_start(wk_sb[:], wkt.rearrange("(c p) d -> p c d", p=P))
        nc.sync.dma_start(bk_sb[:], bk.rearrange("(c p) o -> p c o", p=P))
        ones_f32 = consts.tile([1, P], F32, tag="ones_f32")
        nc.vector.memset(ones_f32[:], 1.0)
        ones_row = consts.tile([1, P], F32R, tag="ones_row")
        nc.vector.tensor_copy(ones_row[:], ones_f32[:])

        # ---- persistent activations ----
        qt_sb = qkv.tile([P, DC // P, S], BF16, tag="qt")
        kt_sb = qkv.tile([P, DC // P, S], BF16, tag="kt")
        # v natural [s, d] + ones col per head: [P, sblk, head, 65]
        v_sb = qkv.tile([P, SB, HPC, DK + 1], BF16, tag="v")
        nc.vector.memset(v_sb[:, :, :, DK:], 1.0)
        outt_sb = qkv.tile([P, 2, S], BF16, tag="outt")  # [d-in-pair, pair, q]

        def scores_exp(pair, qs, pt):
            """S^T matmuls (row-packed head pair) + exp into pt."""
            q0 = qs * QSUP
            for kb in range(SB):
                sc_ps = sc_pool.tile([P, 2, QSUP], F32, tag="sc", name="sc")
                for hh in range(2):          # head-in-pair -> partitions 64*hh
                    hp = hh * DK
                    for qc in range(QC):
                        nc.tensor.matmul(
                            sc_ps[:, hh, qc * 512:(qc + 1) * 512],
                            kt_sb[hp:hp + DK, pair, kb * P:(kb + 1) * P],
                            qt_sb[hp:hp + DK, pair,
                                  q0 + qc * 512:q0 + (qc + 1) * 512],
                            start=True, stop=True,
                        )
                nc.scalar.activation(
                    pt[:, :, kb, :], sc_ps[:],
                    mybir.ActivationFunctionType.Exp,
                    bias=0.0, scale=0.125,
                )

        def av_normalize(pair, qs, pt):
            """A·V ([v|1]: row 64 = denominator), then scale by 1/den."""
            q0 = qs * QSUP
            av_ps = [
                att_ps.tile([DK + 1, QSUP], F32, tag=f"av{hh}", name=f"av{hh}")
                for hh in range(2)
            ]
            for kb in range(SB):
                for hh in range(2):
                    h = 2 * pair + hh
                    for qc in range(QC):
                        nc.tensor.matmul(
                            av_ps[hh][:, qc * 512:(qc + 1) * 512],
                            v_sb[:, kb, h, :],
                            pt[:, hh, kb, qc * 512:(qc + 1) * 512],
                            start=(kb == 0), stop=(kb == SB - 1),
                        )
            # reciprocal denominator, replicated across partitions via a
            # K=1 ones matmul (f32r runs at full PE rate for N>=256)
            rec = dyn.tile([1, 2, QSUP], F32, tag="rec", name="rec")
            rec_r = dyn.tile([1, 2, QSUP], F32R, tag="rec_r", name="rec_r")
            rec_ps = sc_pool.tile([P, 2, QSUP], F32, tag="sc", name="rec_ps")
            for hh in range(2):
                nc.vector.reciprocal(rec[:, hh, :], av_ps[hh][DK:, :])
                nc.vector.tensor_copy(rec_r[:, hh, :], rec[:, hh, :])
                nc.tensor.matmul(
                    rec_ps[:, hh, :], ones_row[:], rec_r[:, hh, :],
                    start=True, stop=True,
                )
            rec_rep = dyn.tile([P, 2, QSUP], F32, tag="rec_rep", name="rec_rep")
            nc.vector.tensor_copy(rec_rep[:], rec_ps[:])
            for hh in range(2):
                nc.vector.tensor_tensor(
                    outt_sb[hh * DK:(hh + 1) * DK, pair, q0:q0 + QSUP],
                    av_ps[hh][:DK, :],
                    rec_rep[hh * DK:(hh + 1) * DK, hh, :],
                    mybir.AluOpType.mult,
                )

        def o_proj(qs):
            """Output projection + store for the s-range covered by qs."""
            for sb_i in range(qs * QSUP // P, (qs + 1) * QSUP // P):
                for ncnk in range(D // 512):
                    ps = opr_ps.tile([P, 512], F32, tag="oproj_ps", name="ps")
                    for pair in range(2):
                        nc.tensor.matmul(
                            ps[:],
                            outt_sb[:, pair, sb_i * P:(sb_i + 1) * P],
                            wo_sb[:, pair, ncnk * 512:(ncnk + 1) * 512],
                            start=(pair == 0), stop=(pair == 1),
                        )
                    o_sb = opool.tile([P, 512], F32, tag="o_out", name="o_sb")
                    nc.vector.tensor_copy(o_sb[:], ps[:])
                    nc.sync.dma_start(
                        outp[sb_i * P:(sb_i + 1) * P,
                             ncnk * 512:(ncnk + 1) * 512],
                        o_sb[:],
                    )

        def qk_proj(x_sb, w_sb, b_sb, t_sb, dc):
            for ss in range(S // 512):
                ps = psp.tile([P, 512], F32, tag="proj_ps", name="ps")
                for fc in range(FC):
                    nc.tensor.matmul(
                        ps[:],
                        w_sb[:, fc, dc * P:(dc + 1) * P],
                        x_sb[:, fc, ss * 512:(ss + 1) * 512],
                        start=(fc == 0), stop=(fc == FC - 1),
                    )
                nc.vector.tensor_scalar(
                    t_sb[:, dc, ss * 512:(ss + 1) * 512],
                    ps[:], b_sb[:, dc, :], None, mybir.AluOpType.add,
                )

        # ---- phase A: load X_k/X_q (chunked), project k/q, and emit the
        # first two attention blocks (pair 0 only needs d-chunk 0) so
        # ScalarE has ~37us of exp work covering the remaining projections
        with tc.tile_pool(name="xt_kq", bufs=1) as xt_kq:
            xk_sb = xt_kq.tile([P, FC, S], BF16, tag="xk")
            xq_sb = xt_kq.tile([P, FC, S], BF16, tag="xq")
            srck = xtk.rearrange("(c p) s -> p c s", p=P)
            for fc in range(FC):
                nc.sync.dma_start(xk_sb[:, fc, :], srck[:, fc, :])
            nc.sync.dma_start(wq_sb[:], wqt.rearrange("(c p) d -> p c d", p=P))
            nc.sync.dma_start(bq_sb[:], bq.rearrange("(c p) o -> p c o", p=P))
            srcq = xtq.rearrange("(c p) s -> p c s", p=P)
            for fc in range(FC):
                nc.sync.dma_start(xq_sb[:, fc, :], srcq[:, fc, :])
            nc.sync.dma_start(wo_sb[:], wot.rearrange("(c p) n -> p c n", p=P))

            qk_proj(xk_sb, wk_sb, bk_sb, kt_sb, 0)
            qk_proj(xq_sb, wq_sb, bq_sb, qt_sb, 0)
            pt0 = ptpool.tile([P, 2, SB, QSUP], BF16, tag="pt", name="pt0")
            scores_exp(0, 0, pt0)
            pt1 = ptpool.tile([P, 2, SB, QSUP], BF16, tag="pt", name="pt1")
            scores_exp(0, 1, pt1)
            qk_proj(xk_sb, wk_sb, bk_sb, kt_sb, 1)
            qk_proj(xq_sb, wq_sb, bq_sb, qt_sb, 1)

        # ---- phase B: load X_v, project v (+bias via K=1 broadcast) ----
        with tc.tile_pool(name="xt_v", bufs=1) as xt_v:
            wv_sb = xt_v.tile([P, FC, DC], BF16, tag="wv")
            nc.sync.dma_start(wv_sb[:], wvt.rearrange("(c p) d -> p c d", p=P))
            bv_row = xt_v.tile([1, DC], F32, tag="bvrow")
            nc.sync.dma_start(bv_row[:], bvr[:])
            bv_r = xt_v.tile([1, DC], F32R, tag="bv_r")
            nc.vector.tensor_copy(bv_r[:], bv_row[:])
            bv_rep = xt_v.tile([P, DC], F32, tag="bvrep")
            bv_ps = psp.tile([P, DC], F32, tag="vproj_ps")
            nc.tensor.matmul(bv_ps[:], ones_row[:], bv_r[:], start=True, stop=True)
            nc.vector.tensor_copy(bv_rep[:], bv_ps[:])

            xv_sb = xt_v.tile([P, FC, S], BF16, tag="xv")
            srcv = xtv.rearrange("(c p) s -> p c s", p=P)
            for fc in range(FC):
                nc.sync.dma_start(xv_sb[:, fc, :], srcv[:, fc, :])
            for sb_i in range(SB):
                ps = psp.tile([P, DC], F32, tag="vproj_ps", name="ps")
                for fc in range(FC):
                    nc.tensor.matmul(
                        ps[:],
                        xv_sb[:, fc, sb_i * P:(sb_i + 1) * P],
                        wv_sb[:, fc, :],
                        start=(fc == 0), stop=(fc == FC - 1),
                    )
                nc.vector.tensor_tensor(
                    v_sb[:, sb_i, :, :DK],
                    ps[:].rearrange("p (h d) -> p h d", h=HPC),
                    bv_rep[:].rearrange("p (h d) -> p h d", h=HPC),
                    mybir.AluOpType.add,
                )
            # drain pair (0, qs=0) inside this phase so the pt slot frees
            # before the main loop's first scores block needs it
            av_normalize(0, 0, pt0)

        psp.release()
        opr_ps = tc.alloc_tile_pool(name="opr_ps", bufs=2, space="PSUM")

        # ---- main pipeline: blocks (0,0) and (0,1) already emitted ----
        # iteration order: pair-0 first two (only need d-chunk 0), then
        # interleaved so each qs's O-projection fires as soon as both pairs
        # finish. pending holds blocks whose A·V tail is not yet emitted.
        rest = [(0, 1), (1, 1), (2, 0), (2, 1), (3, 0), (3, 1)]
        pending = [(1, 0, pt1)]
        done_pairs = {(0, 0): True}  # A·V emitted at the end of phase B
        for qs, pair in rest:
            pt = ptpool.tile([P, 2, SB, QSUP], BF16, tag="pt", name="pt")
            scores_exp(pair, qs, pt)
            pqs, ppair, ppt = pending.pop(0)
            av_normalize(ppair, pqs, ppt)
            done_pairs[(pqs, ppair)] = True
            if (pqs, 0) in done_pairs and (pqs, 1) in done_pairs:
                o_proj(pqs)
            pending.append((qs, pair, pt))
        for pqs, ppair, ppt in pending:
            av_normalize(ppair, pqs, ppt)
            done_pairs[(pqs, ppair)] = True
            if (pqs, 0) in done_pairs and (pqs, 1) in done_pairs:
                o_proj(pqs)
        for pool in (opr_ps, att_ps, sc_pool, opool, dyn, ptpool, qkv, consts):
            pool.release()  # LIFO within each memory space

    n_split = split_multi_waits(nc)
    return nc


_NC_CACHE = None


def prep_in_maps(Q, K, V, W_q, b_q, W_k, b_k, W_v, b_v, W_o, b_o):
    """Host-side sharding: per-core input dicts (transposed, bf16-cast)."""
    bf = ml_dtypes.bfloat16
    Q, K, V = np.asarray(Q), np.asarray(K), np.asarray(V)
    xt = {}   # per batch: transposed bf16 inputs
    for b in range(B):
        xt[b] = (
            np.ascontiguousarray(Q[b].T).astype(bf),
            np.ascontiguousarray(K[b].T).astype(bf),
            np.ascontiguousarray(V[b].T).astype(bf),
        )
    in_maps = []
    for c in range(N_CORES):
        b = c // 4
        g = c % 4
        sl = slice(g * DC, (g + 1) * DC)
        in_maps.append({
            "xtq": xt[b][0], "xtk": xt[b][1], "xtv": xt[b][2],
            "wqt": np.ascontiguousarray(np.asarray(W_q)[sl, :].T).astype(bf),
            "wkt": np.ascontiguousarray(np.asarray(W_k)[sl, :].T).astype(bf),
            "wvt": np.ascontiguousarray(np.asarray(W_v)[sl, :].T).astype(bf),
            "wot": np.ascontiguousarray(np.asarray(W_o)[:, sl].T).astype(bf),
            "bq": np.asarray(b_q)[sl].reshape(DC, 1).astype(np.float32),
            "bk": np.asarray(b_k)[sl].reshape(DC, 1).astype(np.float32),
            "bvr": np.asarray(b_v)[sl].reshape(1, DC).astype(np.float32),
        })
    return in_maps


def gather_out(partials, b_o):
    """Host-side unshard: sum the four W_o-row partials per batch + b_o."""
    out = np.zeros((B, S, D), np.float32)
    for c in range(N_CORES):
        out[c // 4] += partials[c]
    out += np.asarray(b_o).astype(np.float32)
    return out


def kernel(Q, K, V, W_q, b_q, W_k, b_k, W_v, b_v, W_o, b_o):
    global _NC_CACHE
    in_maps = prep_in_maps(Q, K, V, W_q, b_q, W_k, b_k, W_v, b_v, W_o, b_o)
    if _NC_CACHE is None:
        _NC_CACHE = build_bass()
    res = run_bass_kernel_spmd(_NC_CACHE, in_maps, core_ids=list(range(N_CORES)))
    return gather_out([res.results[c]["outp"] for c in range(N_CORES)], b_o)


# revision 23
# speedup vs baseline: 36089.4587x; 108.4160x over previous
"""Multi-head attention (B=2, S=2048, D=1024, H=16) on 8 TRN2 NeuronCores.

Sharding: core c handles batch c//4 and heads 4*(c%4) .. 4*(c%4)+4
(tensor-parallel over heads x data-parallel over batch).

Per-core pipeline (all matmuls bf16 with fp32 PSUM accumulation):
  1. qT/kT = W @ X^T   [d=256 on partitions, s=2048 free]  (transposed proj)
     v     = X @ W^T   [s on partitions, d free] + ones column (for softmax den)
  2. Per head pair (row-packed on the PE array, contraction d=64):
     S^T[k,q] = kT.T @ qT   -> PSUM, exp(S/8) on ScalarE -> p^T bf16 in SBUF
  3. out^T[d,q] = sum_k [v|1].T @ p^T  (M=65: row 64 = softmax denominator)
     scale by the reciprocal denominator (replicated across partitions via a
     K=1 ones matmul in f32r; DVE multiplies it into the A-V psum)
  4. partial[s, 1024] = out^T.T @ WoT  (only this core's head block of W_o)
Host: full output[b] = sum of the 4 partials for batch b + b_o.
"""
import numpy as np
import ml_dtypes

import concourse.bass as bass
import concourse.mybir as mybir
from concourse.tile import TileContext
from concourse.bass_utils import run_bass_kernel_spmd


def split_multi_waits(nc):
    """This container's walrus codegen allows only one sync-wait command per
    instruction ("Too many sync wait commands" in setupSyncWait). Tile
    sometimes attaches several semaphore waits to one instruction; hoist the
    extras onto dedicated EventSemaphore instructions inserted immediately
    before, on the same engine (sequencers execute in order, so semantics
    are identical)."""
    n = [0]
    for f in nc.m.functions:
        for blk in f.blocks:
            new_insts = []
            changed = False
            for inst in blk.instructions:
                si = inst.sync_info
                if si is not None and len(si.on_wait) > 1:
                    waits = list(si.on_wait)
                    for w in waits[:-1]:
                        n[0] += 1
                        ev = mybir.InstEventSemaphore(
                            name=f"WSPLIT-{n[0]}",
                            ins=[], outs=[],
                        )
                        ev.engine = inst.engine
                        ev.sync_info = mybir.SyncInfo(on_wait=[w], on_update=[])
                        new_insts.append(ev)
                        nc.register_instruction(ev, overwrite=True)
                    inst.sync_info = mybir.SyncInfo(
                        on_wait=[waits[-1]], on_update=list(si.on_update)
                    )
                    changed = True
                new_insts.append(inst)
            if changed:
                blk.instructions = new_insts
    return n[0]

BF16 = mybir.dt.bfloat16
F32 = mybir.dt.float32
F32R = mybir.dt.float32r

B, S, D = 2, 2048, 1024
H, DK = 16, 64
HPC = 4              # heads per core
DC = HPC * DK        # 256 d-model dims per core
N_CORES = 8
P = 128              # partitions
SB = S // P          # 16 s-blocks
FC = D // P          # 8 feature chunks
QSUP = 512           # q tile width for attention
NQ = S // QSUP       # 2 q supertiles
QC = QSUP // 512     # 512-wide matmul chunks per q supertile


def build_bass():
    nc = bass.Bass()
    # DRAM inputs (per-core shard, prepared on host)
    xtq = nc.dram_tensor("xtq", [D, S], BF16, kind="ExternalInput")
    xtk = nc.dram_tensor("xtk", [D, S], BF16, kind="ExternalInput")
    xtv = nc.dram_tensor("xtv", [D, S], BF16, kind="ExternalInput")
    wqt = nc.dram_tensor("wqt", [D, DC], BF16, kind="ExternalInput")
    wkt = nc.dram_tensor("wkt", [D, DC], BF16, kind="ExternalInput")
    wvt = nc.dram_tensor("wvt", [D, DC], BF16, kind="ExternalInput")
    wot = nc.dram_tensor("wot", [DC, D], BF16, kind="ExternalInput")
    bq = nc.dram_tensor("bq", [DC, 1], F32, kind="ExternalInput")
    bk = nc.dram_tensor("bk", [DC, 1], F32, kind="ExternalInput")
    bvr = nc.dram_tensor("bvr", [1, DC], F32, kind="ExternalInput")
    outp = nc.dram_tensor("outp", [S, D], F32, kind="ExternalOutput")

    with TileContext(nc) as tc:
        # SBUF pool stack (LIFO releases): consts, qkv, ptpool | xt_v, xt_kq
        # PSUM pool stack: sc_pool, att_ps | psp, then opr_ps
        consts = tc.alloc_tile_pool(name="consts", bufs=1)
        qkv = tc.alloc_tile_pool(name="qkv", bufs=1)
        ptpool = tc.alloc_tile_pool(name="ptpool", bufs=2)
        dyn = tc.alloc_tile_pool(name="dyn", bufs=1)
        opool = tc.alloc_tile_pool(name="opool", bufs=2)
        sc_pool = tc.alloc_tile_pool(name="sc_pool", bufs=2, space="PSUM")
        att_ps = tc.alloc_tile_pool(name="att_ps", bufs=1, space="PSUM")
        psp = tc.alloc_tile_pool(name="psp", bufs=1, space="PSUM")

        # ---- constants (k path first: it gates the critical path) ----
        wk_sb = consts.tile([P, FC, DC], BF16, tag="wk")
        wq_sb = consts.tile([P, FC, DC], BF16, tag="wq")
        bq_sb = consts.tile([P, DC // P, 1], F32, tag="bq")
        bk_sb = consts.tile([P, DC // P, 1], F32, tag="bk")
        wo_sb = consts.tile([P, DC // P, D], BF16, tag="wo")
        nc.sync.dma_start(wk_sb[:], wkt.rearrange("(c p) d -> p c d", p=P))
        nc.sync.dma_start(bk_sb[:], bk.rearrange("(c p) o -> p c o", p=P))
        ones_f32 = consts.tile([1, P], F32, tag="ones_f32")
        nc.vector.memset(ones_f32[:], 1.0)
        ones_row = consts.tile([1, P], F32R, tag="ones_row")
        nc.vector.tensor_copy(ones_row[:], ones_f32[:])

        # ---- persistent activations ----
        qt_sb = qkv.tile([P, DC // P, S], BF16, tag="qt")
        kt_sb = qkv.tile([P, DC // P, S], BF16, tag="kt")
        # v natural [s, d] + ones col per head: [P, sblk, head, 65]
        v_sb = qkv.tile([P, SB, HPC, DK + 1], BF16, tag="v")
        nc.vector.memset(v_sb[:, :, :, DK:], 1.0)
        outt_sb = qkv.tile([P, 2, S], BF16, tag="outt")  # [d-in-pair, pair, q]

        def scores_exp(pair, qs, pt):
            """S^T matmuls (row-packed head pair) + exp into pt."""
            q0 = qs * QSUP
            for kb in range(SB):
                sc_ps = sc_pool.tile([P, 2, QSUP], F32, tag="sc", name="sc")
                for hh in range(2):          # head-in-pair -> partitions 64*hh
                    hp = hh * DK
                    for qc in range(QC):
                        nc.tensor.matmul(
                            sc_ps[:, hh, qc * 512:(qc + 1) * 512],
                            kt_sb[hp:hp + DK, pair, kb * P:(kb + 1) * P],
                            qt_sb[hp:hp + DK, pair,
                                  q0 + qc * 512:q0 + (qc + 1) * 512],
                            start=True, stop=True,
                        )
                nc.scalar.activation(
                    pt[:, :, kb, :], sc_ps[:],
                    mybir.ActivationFunctionType.Exp,
                    bias=0.0, scale=0.125,
                )

        def av_alloc():
            return [
                att_ps.tile([DK + 1, QSUP], F32, tag=f"av{hh}", name=f"av{hh}")
                for hh in range(2)
            ]

        def av_kb(pair, pt, av_ps, kb):
            for hh in range(2):
                h = 2 * pair + hh
                for qc in range(QC):
                    nc.tensor.matmul(
                        av_ps[hh][:, qc * 512:(qc + 1) * 512],
                        v_sb[:, kb, h, :],
                        pt[:, hh, kb, qc * 512:(qc + 1) * 512],
                        start=(kb == 0), stop=(kb == SB - 1),
                    )

        def normalize(pair, qs, av_ps):
            # K=1 ones matmul (f32r runs at full PE rate for N>=256)
            # replicates the reciprocal denominator across partitions
            q0 = qs * QSUP
            rec = dyn.tile([1, 2, QSUP], F32, tag="rec", name="rec")
            rec_r = dyn.tile([1, 2, QSUP], F32R, tag="rec_r", name="rec_r")
            rec_ps = sc_pool.tile([P, 2, QSUP], F32, tag="sc", name="rec_ps")
            for hh in range(2):
                nc.vector.reciprocal(rec[:, hh, :], av_ps[hh][DK:, :])
                nc.vector.tensor_copy(rec_r[:, hh, :], rec[:, hh, :])
                nc.tensor.matmul(
                    rec_ps[:, hh, :], ones_row[:], rec_r[:, hh, :],
                    start=True, stop=True,
                )
            rec_rep = dyn.tile([P, 2, QSUP], F32, tag="rec_rep", name="rec_rep")
            nc.vector.tensor_copy(rec_rep[:], rec_ps[:])
            for hh in range(2):
                nc.vector.tensor_tensor(
                    outt_sb[hh * DK:(hh + 1) * DK, pair, q0:q0 + QSUP],
                    av_ps[hh][:DK, :],
                    rec_rep[hh * DK:(hh + 1) * DK, hh, :],
                    mybir.AluOpType.mult,
                )

        def av_normalize(pair, qs, pt):
            """A·V ([v|1]: row 64 = denominator), then scale by 1/den."""
            av_ps = av_alloc()
            for kb in range(SB):
                av_kb(pair, pt, av_ps, kb)
            normalize(pair, qs, av_ps)

        def o_proj(qs):
            """Output projection + store for the s-range covered by qs."""
            for sb_i in range(qs * QSUP // P, (qs + 1) * QSUP // P):
                for ncnk in range(D // 512):
                    ps = opr_ps.tile([P, 512], F32, tag="oproj_ps", name="ps")
                    for pair in range(2):
                        nc.tensor.matmul(
                            ps[:],
                            outt_sb[:, pair, sb_i * P:(sb_i + 1) * P],
                            wo_sb[:, pair, ncnk * 512:(ncnk + 1) * 512],
                            start=(pair == 0), stop=(pair == 1),
                        )
                    o_sb = opool.tile([P, 512], F32, tag="o_out", name="o_sb")
                    nc.vector.tensor_copy(o_sb[:], ps[:])
                    nc.sync.dma_start(
                        outp[sb_i * P:(sb_i + 1) * P,
                             ncnk * 512:(ncnk + 1) * 512],
                        o_sb[:],
                    )

        def qk_proj(x_sb, w_sb, b_sb, t_sb, dc):
            for ss in range(S // 512):
                ps = psp.tile([P, 512], F32, tag="proj_ps", name="ps")
                for fc in range(FC):
                    nc.tensor.matmul(
                        ps[:],
                        w_sb[:, fc, dc * P:(dc + 1) * P],
                        x_sb[:, fc, ss * 512:(ss + 1) * 512],
                        start=(fc == 0), stop=(fc == FC - 1),
                    )
                nc.vector.tensor_scalar(
                    t_sb[:, dc, ss * 512:(ss + 1) * 512],
                    ps[:], b_sb[:, dc, :], None, mybir.AluOpType.add,
                )

        # ---- phase A: load X_k/X_q (chunked), project k/q, and emit the
        # first two attention blocks (pair 0 only needs d-chunk 0) so
        # ScalarE has ~37us of exp work covering the remaining projections
        with tc.tile_pool(name="xt_kq", bufs=1) as xt_kq:
            xk_sb = xt_kq.tile([P, FC, S], BF16, tag="xk")
            xq_sb = xt_kq.tile([P, FC, S], BF16, tag="xq")
            srck = xtk.rearrange("(c p) s -> p c s", p=P)
            for fc in range(FC):
                nc.sync.dma_start(xk_sb[:, fc, :], srck[:, fc, :])
            nc.sync.dma_start(wq_sb[:], wqt.rearrange("(c p) d -> p c d", p=P))
            nc.sync.dma_start(bq_sb[:], bq.rearrange("(c p) o -> p c o", p=P))
            srcq = xtq.rearrange("(c p) s -> p c s", p=P)
            for fc in range(FC):
                nc.sync.dma_start(xq_sb[:, fc, :], srcq[:, fc, :])
            nc.sync.dma_start(wo_sb[:], wot.rearrange("(c p) n -> p c n", p=P))

            qk_proj(xk_sb, wk_sb, bk_sb, kt_sb, 0)
            qk_proj(xq_sb, wq_sb, bq_sb, qt_sb, 0)
            pt0 = ptpool.tile([P, 2, SB, QSUP], BF16, tag="pt", name="pt0")
            scores_exp(0, 0, pt0)
            pt1 = ptpool.tile([P, 2, SB, QSUP], BF16, tag="pt", name="pt1")
            scores_exp(0, 1, pt1)
            qk_proj(xk_sb, wk_sb, bk_sb, kt_sb, 1)
            qk_proj(xq_sb, wq_sb, bq_sb, qt_sb, 1)

        # ---- phase B: load X_v, project v (+bias via K=1 broadcast) ----
        with tc.tile_pool(name="xt_v", bufs=1) as xt_v:
            wv_sb = xt_v.tile([P, FC, DC], BF16, tag="wv")
            nc.sync.dma_start(wv_sb[:], wvt.rearrange("(c p) d -> p c d", p=P))
            bv_row = xt_v.tile([1, DC], F32, tag="bvrow")
            nc.sync.dma_start(bv_row[:], bvr[:])
            bv_r = xt_v.tile([1, DC], F32R, tag="bv_r")
            nc.vector.tensor_copy(bv_r[:], bv_row[:])
            bv_rep = xt_v.tile([P, DC], F32, tag="bvrep")
            bv_ps = psp.tile([P, DC], F32, tag="vproj_ps")
            nc.tensor.matmul(bv_ps[:], ones_row[:], bv_r[:], start=True, stop=True)
            nc.vector.tensor_copy(bv_rep[:], bv_ps[:])

            xv_sb = xt_v.tile([P, FC, S], BF16, tag="xv")
            srcv = xtv.rearrange("(c p) s -> p c s", p=P)
            for fc in range(FC):
                nc.sync.dma_start(xv_sb[:, fc, :], srcv[:, fc, :])
            for sb_i in range(SB):
                ps = psp.tile([P, DC], F32, tag="vproj_ps", name="ps")
                for fc in range(FC):
                    nc.tensor.matmul(
                        ps[:],
                        xv_sb[:, fc, sb_i * P:(sb_i + 1) * P],
                        wv_sb[:, fc, :],
                        start=(fc == 0), stop=(fc == FC - 1),
                    )
                nc.vector.tensor_tensor(
                    v_sb[:, sb_i, :, :DK],
                    ps[:].rearrange("p (h d) -> p h d", h=HPC),
                    bv_rep[:].rearrange("p (h d) -> p h d", h=HPC),
                    mybir.AluOpType.add,
                )
            # drain pair (0, qs=0) inside this phase so the pt slot frees
            # before the main loop's first scores block needs it
            av_normalize(0, 0, pt0)

        psp.release()
        opr_ps = tc.alloc_tile_pool(name="opr_ps", bufs=2, space="PSUM")

        # ---- main pipeline: blocks (0,0) and (0,1) already emitted ----
        # iteration order: pair-0 first two (only need d-chunk 0), then
        # interleaved so each qs's O-projection fires as soon as both pairs
        # finish. pending holds blocks whose A·V tail is not yet emitted.
        rest = [(0, 1), (1, 1), (2, 0), (2, 1), (3, 0), (3, 1)]
        pending = [(1, 0, pt1)]
        done_pairs = {(0, 0): True}  # A·V emitted at the end of phase B
        for qs, pair in rest:
            pt = ptpool.tile([P, 2, SB, QSUP], BF16, tag="pt", name="pt")
            pqs, ppair, ppt = pending.pop(0)
            av_ps = av_alloc()
            q0 = qs * QSUP
            for kb in range(SB):
                sc_ps = sc_pool.tile([P, 2, QSUP], F32, tag="sc", name="sc")
                for hh in range(2):
                    hp = hh * DK
                    for qc in range(QC):
                        nc.tensor.matmul(
                            sc_ps[:, hh, qc * 512:(qc + 1) * 512],
                            kt_sb[hp:hp + DK, pair, kb * P:(kb + 1) * P],
                            qt_sb[hp:hp + DK, pair,
                                  q0 + qc * 512:q0 + (qc + 1) * 512],
                            start=True, stop=True,
                        )
                nc.scalar.activation(
                    pt[:, :, kb, :], sc_ps[:],
                    mybir.ActivationFunctionType.Exp,
                    bias=0.0, scale=0.125,
                )
                av_kb(ppair, ppt, av_ps, kb)
            normalize(ppair, pqs, av_ps)
            done_pairs[(pqs, ppair)] = True
            if (pqs, 0) in done_pairs and (pqs, 1) in done_pairs:
                o_proj(pqs)
            pending.append((qs, pair, pt))
        for pqs, ppair, ppt in pending:
            av_normalize(ppair, pqs, ppt)
            done_pairs[(pqs, ppair)] = True
            if (pqs, 0) in done_pairs and (pqs, 1) in done_pairs:
                o_proj(pqs)
        for pool in (opr_ps, att_ps, sc_pool, opool, dyn, ptpool, qkv, consts):
            pool.release()  # LIFO within each memory space

    n_split = split_multi_waits(nc)
    return nc


_NC_CACHE = None


def prep_in_maps(Q, K, V, W_q, b_q, W_k, b_k, W_v, b_v, W_o, b_o):
    """Host-side sharding: per-core input dicts (transposed, bf16-cast)."""
    bf = ml_dtypes.bfloat16
    Q, K, V = np.asarray(Q), np.asarray(K), np.asarray(V)
    xt = {}   # per batch: transposed bf16 inputs
    for b in range(B):
        xt[b] = (
            np.ascontiguousarray(Q[b].T).astype(bf),
            np.ascontiguousarray(K[b].T).astype(bf),
            np.ascontiguousarray(V[b].T).astype(bf),
        )
    in_maps = []
    for c in range(N_CORES):
        b = c // 4
        g = c % 4
        sl = slice(g * DC, (g + 1) * DC)
        in_maps.append({
            "xtq": xt[b][0], "xtk": xt[b][1], "xtv": xt[b][2],
            "wqt": np.ascontiguousarray(np.asarray(W_q)[sl, :].T).astype(bf),
            "wkt": np.ascontiguousarray(np.asarray(W_k)[sl, :].T).astype(bf),
            "wvt": np.ascontiguousarray(np.asarray(W_v)[sl, :].T).astype(bf),
            "wot": np.ascontiguousarray(np.asarray(W_o)[:, sl].T).astype(bf),
            "bq": np.asarray(b_q)[sl].reshape(DC, 1).astype(np.float32),
            "bk": np.asarray(b_k)[sl].reshape(DC, 1).astype(np.float32),
            "bvr": np.asarray(b_v)[sl].reshape(1, DC).astype(np.float32),
        })
    return in_maps


def gather_out(partials, b_o):
    """Host-side unshard: sum the four W_o-row partials per batch + b_o."""
    out = np.zeros((B, S, D), np.float32)
    for c in range(N_CORES):
        out[c // 4] += partials[c]
    out += np.asarray(b_o).astype(np.float32)
    return out


def kernel(Q, K, V, W_q, b_q, W_k, b_k, W_v, b_v, W_o, b_o):
    global _NC_CACHE
    in_maps = prep_in_maps(Q, K, V, W_q, b_q, W_k, b_k, W_v, b_v, W_o, b_o)
    if _NC_CACHE is None:
        _NC_CACHE = build_bass()
    res = run_bass_kernel_spmd(_NC_CACHE, in_maps, core_ids=list(range(N_CORES)))
    return gather_out([res.results[c]["outp"] for c in range(N_CORES)], b_o)
